# revision 1
# baseline (speedup 1.0000x reference)
"""JSD loss kernel for Trainium2 (8 NeuronCores, row-sharded SPMD).

Reformulation: with lp3 = lp + (B+1)*ln2 (fp16, B=15), p' = exp(lp3 - ln2)
= p*2^B and m' = p'+q' = m*2^(B+1), the bias cancels inside
diff = lp3 - ln(m') = ln(p/m), so the whole loss is ONE product stream:

  loss_r = 2^-(B+1) * sum_v [ p'*(lp3 - lnm') + q'*(lq3 - lnm') ]

Per 2048-col sub-pair (interleaved [lp_s|lq_s] layout):
  ACT Exp -> pq ; PE identity-matmul -> m' (PSUM) ; ACT Ln (lag 1 sub,
  PSUM freed immediately) ; DVE diffs (TT 2x, in-place) ; then either
  DVE STT-accum (S-chunks) or DVE prod TT + Pool fold tree + DVE tail
  reduce (F-chunks).

The first three stream subs use a two-accumulator "T" form (W-STT with
no Ln dependency) filling the DVE's fill-idle window; the last two use a
"U" form (W via Ln-independent prod + Pool folds, tiny C-red drain).
Engine busy per core: ACT ~174 us (bottleneck; saturated in steady
state). TimelineSim: 219,348 ns.
"""

import sys
from contextlib import ExitStack

import numpy as np

sys.path.insert(0, "/opt/trn_rl_repo")

N, V = 2048, 32000
NCORES = 8
R = N // NCORES  # 256 rows per core
P = 128
NBLK = R // P  # 2 row blocks per core
CHUNKS = [1024, 2048] + [4096] * 6 + [1280, 1024, 1536, 512]  # sum = 32000
SUBW = 2048
# per-sub W-reduction kind: S = one DVE STT; F = DVE prod + Pool fold tree
# 17 subs per block; ~70% F balances Pool (~165us) vs DVE (~168us)
SKINDS = ["T", "T", "T", "F", "S", "F", "F", "F", "S", "F",
          "F", "F", "S", "F", "S", "S", "U", "U"]
BIAS = 15.0
LN2 = 0.6931471805599453
# chunk kinds per block: S = single STT reduction, 2/3 = fold-tree depth
KINDS = ["S", "F3", "F3", "F3", "S", "F3", "F3", "F2", "S", "S", "S"]

_CACHE = {}


def _preload_act_table(nc):
    """Preload the act-func table containing BOTH Exp and Ln so the
    insert_act_table_loads pass never thrashes tables (1283 ns each)."""
    from concourse import mybir
    from concourse.hw_specs import get_activation_tables

    tabs = get_activation_tables(nc.m.arch)
    E = mybir.ActivationFunctionType
    for i, (name, funcs) in enumerate(tabs.items()):
        if E.Exp in funcs and E.Ln in funcs:
            inst = mybir.InstLoadActFuncSet(
                name=nc.get_next_instruction_name(),
                ins=[],
                outs=[],
                act_func_set_id=i,
            )
            inst.engine = mybir.EngineType.Activation
            nc.scalar.add_instruction(inst)
            return


def _build_program():
    import concourse.bacc as bacc
    import concourse.tile as tile
    from concourse import mybir

    nc = bacc.Bacc(
        "TRN2",
        target_bir_lowering=False,
        debug=False,
        enable_asserts=False,
        num_devices=1,
    )
    lp_d = nc.dram_tensor("lp3", [R, V], mybir.dt.float16, kind="ExternalInput")
    lq_d = nc.dram_tensor("lq3", [R, V], mybir.dt.float16, kind="ExternalInput")
    id_d = nc.dram_tensor("ident", [P, P], mybir.dt.float16, kind="ExternalInput")
    out_d = nc.dram_tensor("loss", [R, 1], mybir.dt.float32, kind="ExternalOutput")

    lp = lp_d.ap()
    lq = lq_d.ap()
    out = out_d.ap()

    fp32 = mybir.dt.float32
    fp16 = mybir.dt.float16
    Exp = mybir.ActivationFunctionType.Exp
    Ln = mybir.ActivationFunctionType.Ln
    mult = mybir.AluOpType.mult
    add = mybir.AluOpType.add
    subtract = mybir.AluOpType.subtract

    CMAX = max(CHUNKS)

    with tile.TileContext(nc) as tc, ExitStack() as ctx:
        _preload_act_table(nc)
        const = ctx.enter_context(tc.tile_pool(name="const", bufs=1))
        loads = ctx.enter_context(tc.tile_pool(name="loads", bufs=9))
        acts = ctx.enter_context(tc.tile_pool(name="acts", bufs=9))
        logms = ctx.enter_context(tc.tile_pool(name="logms", bufs=6))
        folds = ctx.enter_context(tc.tile_pool(name="folds", bufs=3))
        parts = ctx.enter_context(tc.tile_pool(name="parts", bufs=2))
        outs = ctx.enter_context(tc.tile_pool(name="outs", bufs=2))
        psum = ctx.enter_context(tc.tile_pool(name="psum", bufs=2, space="PSUM"))

        ident_sb = const.tile([P, P], fp16)
        nc.sync.dma_start(out=ident_sb[:], in_=id_d.ap())
        neg_ln2 = const.tile([P, 1], fp32)
        nc.vector.memset(neg_ln2[:], -LN2)

        finales = []
        # interleave the two row-blocks chunk-by-chunk: one continuous
        # pipeline, no block transition, single fill + single tail
        ncols = sum((c + SUBW - 1) // SUBW for c in CHUNKS)
        blk = []
        n_t = sum(1 for k in SKINDS if k in ("T", "U"))
        for b in range(NBLK):
            p_parts = parts.tile([P, ncols], fp32, tag="pp", name="p_parts")
            m_parts = parts.tile([P, n_t], fp32, tag="mp", name="m_parts")
            blk.append({
                "r0": b * P, "pcol": 0, "parts": p_parts, "mparts": m_parts,
                "mcol": 0, "gsub": 0, "c0": 0,
            })
            finales.append((p_parts, m_parts, b * P))
        ln_q = []
        diff_q = []
        tail_q = []

        def emit_ln(sub):
            m_, logm_, w_ = sub["m"], sub["logm"], sub["w"]
            nc.scalar.activation(
                out=logm_[:, 0:w_], in_=m_[:, 0:w_], func=Ln
            )
            diff_q.append(sub)

        def emit_dve(sub):
            B_ = sub["blk"]
            lplq_, pq_, logm_ = sub["lplq"], sub["pq"], sub["logm"]
            w_ = sub["w"]
            p_parts = B_["parts"]
            if sub["kind"] in ("T", "U"):
                # two-accum form: W already ran at Exp time (no Ln
                # dep); here only M += sum m'*ln(m')  (PSUM f32 x fp16)
                m_ = sub["m"]
                nc.vector.scalar_tensor_tensor(
                    out=m_[:, 0:w_],
                    in0=m_[:, 0:w_],
                    scalar=1.0,
                    in1=logm_[:, 0:w_],
                    op0=mult,
                    op1=mult,
                    accum_out=B_["mparts"][:, B_["mcol"] : B_["mcol"] + 1],
                )
                B_["mcol"] += 1
                return
            # diff = lp3 - lnm' (= ln(p/m)), in-place over lplq, one TT 2x
            # with lnm broadcast over the [lp_s|lq_s] pair (zero-stride dim)
            pair = lplq_[:, 0 : 2 * w_].rearrange("p (a b) -> p a b", a=2)
            lnb = logm_[:, 0:w_].unsqueeze(1).broadcast_to([P, 2, w_])
            nc.vector.tensor_tensor(out=pair, in0=pair, in1=lnb, op=subtract)
            if sub["kind"] == "S":
                nc.vector.scalar_tensor_tensor(
                    out=lplq_[:, 0 : 2 * w_],
                    in0=lplq_[:, 0 : 2 * w_],
                    scalar=1.0,
                    in1=pq_[:, 0 : 2 * w_],
                    op0=mult,
                    op1=mult,
                    accum_out=p_parts[:, B_["pcol"] : B_["pcol"] + 1],
                )
                B_["pcol"] += 1
            else:
                nc.vector.tensor_tensor(
                    out=lplq_[:, 0 : 2 * w_],
                    in0=lplq_[:, 0 : 2 * w_],
                    in1=pq_[:, 0 : 2 * w_], op=mult,
                )
                fold = folds.tile([P, SUBW], fp32, tag="fold", name="fold")
                nc.gpsimd.tensor_tensor(
                    out=fold[:, 0:w_],
                    in0=lplq_[:, 0:w_],
                    in1=lplq_[:, w_ : 2 * w_], op=add,
                )
                h = w_ // 2
                nc.gpsimd.tensor_tensor(
                    out=fold[:, 0:h], in0=fold[:, 0:h],
                    in1=fold[:, h:w_], op=add,
                )
                h2 = w_ // 4
                nc.gpsimd.tensor_tensor(
                    out=fold[:, 0:h2], in0=fold[:, 0:h2],
                    in1=fold[:, h2:h], op=add,
                )
                tail_q.append((fold, h2, B_, B_["pcol"]))
                B_["pcol"] += 1

        def emit_tail(item):
            fold_, hw_, B_, col = item
            nc.vector.tensor_reduce(
                out=B_["parts"][:, col : col + 1], in_=fold_[:, 0:hw_],
                op=add, axis=mybir.AxisListType.X,
            )

        for i, C in enumerate(CHUNKS):
            for B_ in blk:
                r0 = B_["r0"]
                c0 = B_["c0"]
                nsub = (C + SUBW - 1) // SUBW
                for sidx in range(nsub):
                    j0 = sidx * SUBW
                    w = min(SUBW, C - j0)
                    lplq = loads.tile([P, 2 * SUBW], fp16, tag="lplq",
                                      name="lplq")
                    pq = acts.tile([P, 2 * SUBW], fp16, tag="pq", name="pq")
                    logm = logms.tile([P, SUBW], fp16, tag="logm", name="logm")
                    nc.sync.dma_start(
                        out=lplq[:, 0:w],
                        in_=lp[r0 : r0 + P, c0 + j0 : c0 + j0 + w],
                    )
                    nc.sync.dma_start(
                        out=lplq[:, w : 2 * w],
                        in_=lq[r0 : r0 + P, c0 + j0 : c0 + j0 + w],
                    )
                    while len(ln_q) > 1:
                        emit_ln(ln_q.pop(0))
                    nc.scalar.activation(
                        out=pq[:, 0 : 2 * w],
                        in_=lplq[:, 0 : 2 * w],
                        func=Exp, scale=1.0, bias=neg_ln2[:],
                    )
                    while len(diff_q) > 1:
                        emit_dve(diff_q.pop(0))
                    while len(tail_q) > 1:
                        emit_tail(tail_q.pop(0))
                    m_ps = psum.tile([P, SUBW], fp32, tag="m")
                    for k0 in range(0, w, 512):
                        kw = min(512, w - k0)
                        nc.tensor.matmul(
                            out=m_ps[:, k0 : k0 + kw],
                            lhsT=ident_sb[:],
                            rhs=pq[:, k0 : k0 + kw],
                            start=True,
                            stop=False,
                        )
                        nc.tensor.matmul(
                            out=m_ps[:, k0 : k0 + kw],
                            lhsT=ident_sb[:],
                            rhs=pq[:, w + k0 : w + k0 + kw],
                            start=False,
                            stop=True,
                        )
                    kind = SKINDS[B_["gsub"]]
                    if kind == "U":
                        # W via fold path, Ln-independent: prod in-place,
                        # Pool fold tree (idle at stream end), DVE tail
                        nc.vector.tensor_tensor(
                            out=lplq[:, 0 : 2 * w],
                            in0=lplq[:, 0 : 2 * w],
                            in1=pq[:, 0 : 2 * w], op=mult,
                        )
                        foldu = folds.tile([P, SUBW], fp32, tag="fold",
                                           name="foldu")
                        nc.gpsimd.tensor_tensor(
                            out=foldu[:, 0:w],
                            in0=lplq[:, 0:w],
                            in1=lplq[:, w : 2 * w], op=add,
                        )
                        hu = w // 2
                        nc.gpsimd.tensor_tensor(
                            out=foldu[:, 0:hu], in0=foldu[:, 0:hu],
                            in1=foldu[:, hu:w], op=add,
                        )
                        hu2 = w // 4
                        nc.gpsimd.tensor_tensor(
                            out=foldu[:, 0:hu2], in0=foldu[:, 0:hu2],
                            in1=foldu[:, hu2:hu], op=add,
                        )
                        tail_q.append((foldu, hu2, B_, B_["pcol"]))
                        B_["pcol"] += 1
                    if kind == "T":
                        # W += sum lplq*pq right away (no Ln dependency;
                        # lands in the DVE's idle fill window)
                        nc.vector.scalar_tensor_tensor(
                            out=lplq[:, 0 : 2 * w],
                            in0=lplq[:, 0 : 2 * w],
                            scalar=1.0,
                            in1=pq[:, 0 : 2 * w],
                            op0=mult,
                            op1=mult,
                            accum_out=B_["parts"][
                                :, B_["pcol"] : B_["pcol"] + 1
                            ],
                        )
                        B_["pcol"] += 1
                    ln_q.append(
                        {"m": m_ps, "logm": logm, "lplq": lplq, "pq": pq,
                         "w": w, "kind": kind, "blk": B_}
                    )
                    B_["gsub"] += 1
                B_["c0"] += C
        while ln_q:
            emit_ln(ln_q.pop(0))
        while diff_q:
            emit_dve(diff_q.pop(0))
        while tail_q:
            emit_tail(tail_q.pop(0))
        for B_ in blk:
            assert B_["pcol"] == ncols

        for p_parts, m_parts, r0 in finales:
            loss_b = outs.tile([P, 1], fp32, tag="lb", name="loss_b")
            msum = outs.tile([P, 1], fp32, tag="msb", name="msum")
            nc.vector.reduce_sum(
                out=loss_b[:], in_=p_parts[:], axis=mybir.AxisListType.X
            )
            nc.vector.reduce_sum(
                out=msum[:], in_=m_parts[:], axis=mybir.AxisListType.X
            )
            nc.vector.tensor_tensor(
                out=loss_b[:], in0=loss_b[:], in1=msum[:], op=subtract
            )
            nc.vector.tensor_scalar_mul(
                out=loss_b[:], in0=loss_b[:], scalar1=2.0 ** -(BIAS + 1)
            )
            nc.sync.dma_start(out=out[r0 : r0 + P, :], in_=loss_b[:])

    nc.compile()
    return nc


def _get_program():
    if "nc" not in _CACHE:
        _CACHE["nc"] = _build_program()
    return _CACHE["nc"]


def kernel(log_q: np.ndarray, log_p: np.ndarray, _trace: bool = False):
    from concourse.bass_utils import run_bass_kernel_spmd

    log_q = np.asarray(log_q, dtype=np.float32)
    log_p = np.asarray(log_p, dtype=np.float32)
    assert log_q.shape == (N, V) and log_p.shape == (N, V)

    lp3 = (log_p + (BIAS + 1.0) * LN2).astype(np.float16)
    lq3 = (log_q + (BIAS + 1.0) * LN2).astype(np.float16)

    nc = _get_program()
    ident = np.eye(P, dtype=np.float16)
    in_maps = []
    for c in range(NCORES):
        sl = slice(c * R, (c + 1) * R)
        in_maps.append(
            {"lp3": np.ascontiguousarray(lp3[sl]),
             "lq3": np.ascontiguousarray(lq3[sl]),
             "ident": ident}
        )
    res = run_bass_kernel_spmd(
        nc, in_maps, core_ids=list(range(NCORES)), trace=_trace
    )
    _CACHE["last_results"] = res
    outs = [res.results[c]["loss"].reshape(R) for c in range(NCORES)]
    return np.concatenate(outs, axis=0).astype(np.float32)

